# revision 7
# baseline (speedup 1.0000x reference)
"""BiDAF attention kernel for Trainium2, data-parallel over batch on 8 NeuronCores.

Math (per batch b, with w = [wc; wq; wm]):
    sim[i,j] = c_i@wc + q_j@wq + (c_i*wm)@q_j  =  cb_i + qb_j + s'[i,j]
    c2q      = softmax_j(sim) @ q
    q2c      = softmax_i(max_j sim) @ c            (broadcast over i)

Device strategy (softmax is shift-invariant per row; |sim| <~ 12 so no
max-subtraction is needed in fp32 range):
  - simT[j,i] = sum_d kT[d,j] * cT[d,i] via fp16 matmuls (fp32 PSUM), with
    kT = (q*wm)^T and cT = c^T prepacked on the host. The i columns are
    permuted (u <-> i = (u%128)*8 + u//128) so that the c2q output lands in
    natural [Lc, D] row-major layout with 4KB-contiguous DMA lines.
  - ET[j, jt, u] = exp(simT + qb[j]) via one ACT pass per j-chunk (qb is a
    per-partition bias).  softmax_j(sim)[i,j] = ET[j,u(i)] / S_i exactly
    (the cb_i shift cancels).
  - c2q + row sums S in one matmul chain per i-chunk:
    [c2q_unnorm | S] = ET.T @ [q | 1]; normalize on DVE (recip + scalar mul).
  - sim(b) and c2q(b-1) are interleaved chunk-by-chunk so the tensor engine
    never waits for the ACT exp chain.
  - row maxes: DVE max-tree over the 8 j-chunks -> mx[j, u]; mx is shipped to
    the host, which finishes max-over-j, q2c and the broadcast (tiny work).
"""

import numpy as np

B, LC, LQ, D = 16, 1024, 1024, 256
N_CORES = 8
BPC = B // N_CORES  # batches per core
NJ = LQ // 128      # 8 j-chunks
NI = LC // 128      # 8 i-chunks
ND = D // 128       # 2 d-chunks

# dtype for everything 16-bit ("bf16" or "fp16")
_DT16 = "fp16"

_CACHE = {}


def build_program(repeat_inner=1, n_cores=N_CORES):
    """Build + compile the SPMD bass program (one core's view, BPC batches).

    repeat_inner > 1 repeats the whole body (for timing amplification);
    the pipeline is flat across all repeat_inner*BPC batches."""
    import concourse.bacc as bacc
    import concourse.tile as tile
    from concourse import mybir

    f32 = mybir.dt.float32
    dt16 = mybir.dt.bfloat16 if _DT16 == "bf16" else mybir.dt.float16

    nc = bacc.Bacc(
        "TRN2",
        target_bir_lowering=False,
        debug=False,
        enable_asserts=False,
        num_devices=n_cores,
    )

    # DRAM I/O (per-core shapes); all prepacked host-side for 2-4KB DMA lines
    ct_d = nc.dram_tensor("ct", [BPC, 128, ND, LC], dt16, kind="ExternalInput").ap()
    kt_d = nc.dram_tensor("kt", [BPC, 128, ND, LQ], dt16, kind="ExternalInput").ap()
    qa_d = nc.dram_tensor("qa", [BPC, 128, NJ, D + 2], dt16, kind="ExternalInput").ap()
    qb_d = nc.dram_tensor("qb", [BPC, 128, NJ], f32, kind="ExternalInput").ap()

    c2q_d = nc.dram_tensor("c2q", [BPC, 128, NI, D], dt16, kind="ExternalOutput").ap()
    mx_d = nc.dram_tensor("mx", [BPC, 128, LC], dt16, kind="ExternalOutput").ap()

    with tile.TileContext(nc) as tc:
        with (
            tc.tile_pool(name="io", bufs=2) as io_pool,
            tc.tile_pool(name="et", bufs=2) as et_pool,
            tc.tile_pool(name="mx", bufs=2) as mx_pool,
            tc.tile_pool(name="small", bufs=4) as small_pool,
            tc.tile_pool(name="outs", bufs=2) as out_pool,
            tc.tile_pool(name="psum_sim", bufs=2, space="PSUM") as sim_pool,
            tc.tile_pool(name="psum_c2q", bufs=3, space="PSUM") as c2q_pool,
        ):
            n_batches = repeat_inner * BPC
            prev = None  # (ET, qa_s, c2qo, b) of the previous batch
            for idx in range(n_batches + 1):
                if idx < n_batches:
                    b = idx % BPC
                    # ---- load this batch's inputs (one DMA per tensor) ----
                    kt_s = io_pool.tile([128, ND, LQ], dt16, tag="kt")
                    nc.sync.dma_start(kt_s[:], kt_d[b])
                    ct_s = io_pool.tile([128, ND, LC], dt16, tag="ct")
                    nc.sync.dma_start(ct_s[:], ct_d[b])
                    qb_s = io_pool.tile([128, NJ], f32, tag="qb")
                    nc.sync.dma_start(qb_s[:], qb_d[b])
                    qa_s = io_pool.tile([128, NJ, D + 2], dt16, tag="qa")
                    nc.sync.dma_start(qa_s[:], qa_d[b])
                    ET = et_pool.tile([128, NJ, LQ], dt16, tag="et")
                    mx_s = mx_pool.tile([128, LC], dt16, tag="mx")
                    c2qo = out_pool.tile([128, NI, D], dt16, tag="c2qo")
                else:
                    ET = mx_s = c2qo = None

                for step in range(NJ):
                    if idx < n_batches:
                        # ---- sim chunk jt=step: simT[j, u] + exp ----
                        jt = step
                        ps = sim_pool.tile([128, LC], mybir.dt.float32, tag="sim")
                        for nh in range(2):
                            cols = slice(nh * 512, (nh + 1) * 512)
                            for dc in range(ND):
                                nc.tensor.matmul(
                                    ps[:, cols],
                                    lhsT=kt_s[:, dc, jt * 128:(jt + 1) * 128],
                                    rhs=ct_s[:, dc, cols],
                                    start=(dc == 0),
                                    stop=(dc == ND - 1),
                                )
                        nc.scalar.activation(
                            ET[:, jt, :], ps[:],
                            mybir.ActivationFunctionType.Exp,
                            bias=qb_s[:, jt:jt + 1], scale=1.0,
                        )
                        if jt == 1:
                            nc.vector.tensor_max(mx_s[:], ET[:, 0, :], ET[:, 1, :])
                        elif jt > 1:
                            nc.vector.tensor_max(mx_s[:], mx_s[:], ET[:, jt, :])

                    if prev is not None:
                        # ---- c2q chunk ic=step for the previous batch ----
                        ic = step
                        ETp, qap, c2qop, bp = prev
                        pc = c2q_pool.tile([128, D + 2], mybir.dt.float32, tag="c2q")
                        for jc in range(NJ):
                            nc.tensor.matmul(
                                pc[:],
                                lhsT=ETp[:, jc, ic * 128:(ic + 1) * 128],
                                rhs=qap[:, jc, :],
                                start=(jc == 0),
                                stop=(jc == NJ - 1),
                            )
                        rs = small_pool.tile([128, 1], mybir.dt.float32, tag="recip")
                        nc.vector.reciprocal(rs[:], pc[:, D:D + 1])
                        nc.vector.tensor_scalar_mul(
                            c2qop[:, ic, :], pc[:, 0:D], rs[:])

                if idx < n_batches:
                    nc.sync.dma_start(mx_d[b], mx_s[:])
                if prev is not None:
                    nc.sync.dma_start(c2q_d[prev[3]], prev[2][:])
                prev = (ET, qa_s, c2qo, idx % BPC) if idx < n_batches else None

    nc.compile()
    return nc


def _host_prep(context_features, question_features, weight):
    import ml_dtypes
    np16 = ml_dtypes.bfloat16 if _DT16 == "bf16" else np.float16

    c = np.ascontiguousarray(context_features, dtype=np.float32)
    q = np.ascontiguousarray(question_features, dtype=np.float32)
    w = np.asarray(weight, dtype=np.float32)[:, 0]
    wc, wq, wm = w[:D], w[D:2 * D], w[2 * D:]

    qb = q @ wq                       # [B, LQ]
    cb = c @ wc                       # [B, LC]
    k = q * wm                        # [B, LQ, D]

    # i-column permutation: sim/ET column u holds natural row i = (u%128)*8 + u//128
    u = np.arange(LC)
    i_of_u = (u % 128) * 8 + u // 128

    # ct[b, p, dc, u] = c[b, i_of_u[u], dc*128+p]
    cperm = c[:, i_of_u, :]                                   # [B, LC(u), D]
    ct = np.ascontiguousarray(
        cperm.transpose(0, 2, 1).reshape(B, ND, 128, LC).transpose(0, 2, 1, 3)
    ).astype(np16)                                            # [B, 128, ND, LC]
    # kt[b, p, dc, j] = k[b, j, dc*128+p]
    kt = np.ascontiguousarray(
        k.transpose(0, 2, 1).reshape(B, ND, 128, LQ).transpose(0, 2, 1, 3)
    ).astype(np16)                                            # [B, 128, ND, LQ]
    # qa[b, p, jc, :] = [q[b, jc*128+p, :], 1, 0]
    qa = np.concatenate(
        [q, np.ones((B, LQ, 1), np.float32),
         np.zeros((B, LQ, 1), np.float32)], axis=2)           # [B, LQ, D+2]
    qa = np.ascontiguousarray(
        qa.reshape(B, NJ, 128, D + 2).transpose(0, 2, 1, 3)).astype(np16)
    # qb[b, p, jt] = qb[b, jt*128+p]
    qb_t = np.ascontiguousarray(
        qb.reshape(B, NJ, 128).transpose(0, 2, 1))            # [B, 128, NJ]

    in_maps = []
    for core in range(N_CORES):
        s = slice(core * BPC, (core + 1) * BPC)
        in_maps.append({"ct": ct[s], "kt": kt[s], "qa": qa[s], "qb": qb_t[s]})
    return in_maps, (c, cb, i_of_u)


def _assemble(results, aux):
    c, cb, i_of_u = aux
    # c2q: [B, 128, NI, D] with partition p, chunk ic -> natural row i = p*8+ic
    c2q = np.concatenate(
        [np.asarray(r["c2q"], dtype=np.float32) for r in results], axis=0)
    c2q = c2q.reshape(B, LC, D)
    # mx: [B, 128, LC(u)]; m_u = max_j ET[j, u]; then permute u -> i
    mx = np.concatenate(
        [np.asarray(r["mx"], dtype=np.float32) for r in results], axis=0)
    m_u = mx.max(axis=1)                                      # [B, LC(u)]
    m = np.empty_like(m_u)
    m[:, i_of_u] = m_u                                        # [B, LC(i)]
    e2 = m * np.exp(cb)                                       # exp(max_j sim)
    wq2c = e2 / e2.sum(axis=1, keepdims=True)                 # [B, LC]
    q2c_vec = np.einsum('bi,bid->bd', wq2c, c)                # [B, D]
    q2c = np.broadcast_to(q2c_vec[:, None, :], (B, LC, D)).copy()
    return c2q, q2c


def _make_runner(nc, n_cores):
    """Compile the bass program once into a reusable sharded jax callable."""
    import jax
    import numpy as np
    from jax.sharding import Mesh, PartitionSpec
    from jax.experimental.shard_map import shard_map
    from concourse import mybir
    from concourse.bass2jax import (
        _bass_exec_p, install_neuronx_cc_hook, partition_id_tensor)

    install_neuronx_cc_hook()

    partition_name = nc.partition_id_tensor.name if nc.partition_id_tensor else None
    in_names, out_names, out_avals, zero_shapes = [], [], [], []
    for alloc in nc.m.functions[0].allocations:
        if not isinstance(alloc, mybir.MemoryLocationSet):
            continue
        name = alloc.memorylocations[0].name
        if alloc.kind == "ExternalInput":
            if name != partition_name:
                in_names.append(name)
        elif alloc.kind == "ExternalOutput":
            out_names.append(name)
            shape = tuple(alloc.tensor_shape)
            dtype = mybir.dt.np(alloc.dtype)
            out_avals.append(jax.core.ShapedArray(shape, dtype))
            zero_shapes.append((shape, dtype))
    n_params = len(in_names)
    all_names = list(in_names) + list(out_names)
    if partition_name is not None:
        all_names.append(partition_name)

    def _body(*args):
        operands = list(args)
        if partition_name is not None:
            operands.append(partition_id_tensor())
        outs = _bass_exec_p.bind(
            *operands,
            out_avals=tuple(out_avals),
            in_names=tuple(all_names),
            out_names=tuple(out_names),
            lowering_input_output_aliases=(),
            sim_require_finite=True,
            sim_require_nnan=True,
            nc=nc,
        )
        return tuple(outs)

    devices = jax.devices()[:n_cores]
    assert len(devices) == n_cores, f"need {n_cores} cores"
    mesh = Mesh(np.asarray(devices), ("core",))
    n_outs = len(out_names)
    fn = jax.jit(
        shard_map(
            _body, mesh=mesh,
            in_specs=(PartitionSpec("core"),) * (n_params + n_outs),
            out_specs=(PartitionSpec("core"),) * n_outs,
            check_rep=False),
        keep_unused=True,
    )
    sharding = jax.sharding.NamedSharding(mesh, PartitionSpec("core"))
    zeros = [
        jax.device_put(
            np.zeros((shape[0] * n_cores,) + tuple(shape[1:]), dtype), sharding)
        for shape, dtype in zero_shapes
    ]

    def run(in_maps):
        concat_in = [
            np.concatenate([np.asarray(m[name]) for m in in_maps], axis=0)
            for name in in_names
        ]
        dev_in = [jax.device_put(a, sharding) for a in concat_in]
        outs = fn(*dev_in, *zeros)
        results = []
        for cidx in range(n_cores):
            d = {}
            for name, arr in zip(out_names, outs):
                arr = np.asarray(arr)
                per = arr.shape[0] // n_cores
                d[name] = arr[cidx * per:(cidx + 1) * per]
            results.append(d)
        return results

    return run


def kernel(context_features, question_features, weight):
    if "run" not in _CACHE:
        nc = build_program()
        _CACHE["nc"] = nc
        _CACHE["run"] = _make_runner(nc, N_CORES)

    in_maps, aux = _host_prep(context_features, question_features, weight)
    results = _CACHE["run"](in_maps)
    c2q, q2c = _assemble(results, aux)
    return c2q, q2c
